# revision 10
# baseline (speedup 1.0000x reference)
"""Depthwise causal conv1d (K=4, dilation=1) on 8 TRN2 NeuronCores.

Reference: x [B=8, T=4096, C=1024] f32, W [4, 1, 1024] f32 (WIO layout),
y[b, t, c] = sum_k W[k, 0, c] * x[b, t - 3 + k, c]  (zero left-pad).

Sharding: pure batch data-parallel - core i computes batch i. Memory-bound:
all device I/O is bf16 (tolerance 2e-2; this kernel lands ~6e-3). Host
pre-casts / pre-transposes each batch to [C, T+3] with the causal zero-pad
baked in, so channels sit on SBUF partitions and time shifts are free-dim
offsets.

Layout: one persistent SBUF x tile per 128-channel group (8 x [128, 4099]
bf16 = 66KB/partition), loaded in 2-3 segments per group so early compute
isn't gated on full-group transfers (a transfer's completion semaphore only
fires when all 16 DMA engines finish their slice, so smaller first segments
start the pipeline sooner). Compute is a static plan of [128, cols] pieces:
  A : PE 4 accumulating diag-matmul taps -> PSUM; ScalarE ACTIVATE evicts
      (per 1024-col unit) to bf16.   [PE ~1.75us + ACT ~1.07us per unit]
  B : PE 3 taps -> PSUM; one DVE scalar_tensor_tensor fuses the 4th tap,
      the PSUM add, and the bf16 evict.  [PE ~1.31 + DVE ~1.17 per unit]
  E : ScalarE muls taps 3,2 into temps; DVE ts-muls taps 1,0 + 3 tt-adds.
  D : pure DVE classic (4 ts-mul + 3 tt-add).
A/B pieces up to 2048 cols run tap-outer across per-1024 PSUM units: the
stationary diag block swaps once per tap (matmuls pipeline at ~0.43ns/col)
while evicts drain finished units concurrently (4 psum bufs of 2 banks).
No GpSimd tensor ops: they run ~25x slow and stall concurrent DVE ops.

All x/weight loads ride the in-order sync HWDGE ring, ordered by first
need (diag blocks just-in-time per group); stores go per-unit on the
gpsimd SWDGE ring except the last two pieces, which use the scalar and
sync HWDGE rings (independent drain, both engines idle by then). Dummy
matmuls warm the PE clock before real tiles arrive.
"""

import numpy as np

B, T, C = 8, 4096, 1024
KTAPS = 4
HALO = KTAPS - 1
CG = 128  # channels per partition-group
N_GROUPS = C // CG
N_CORES = 8
MM_N = 512  # matmul moving free dim = one PSUM bank (f32)
PS_N = 1024  # psum/evict/store unit (2 banks)
XCOLS = T + HALO

# module-level stash so test.py can read profiling info
last_results = None


def _default_plan():
    """[(g, c0, cols, path)] in emission order. Unit mix A16/B6/E6/D4
    (1024-col units) -> PE ~37us, Scalar ~30us, DVE ~36us."""
    return [
        (0, 0, 1024, "A"),
        (1, 0, 1024, "D"),
        (0, 1024, 2048, "A"),
        (2, 0, 2048, "E"),
        (3, 0, 2048, "A"),
        (1, 1024, 2048, "B"),
        (0, 3072, 1024, "A"),
        (4, 0, 2048, "E"),
        (2, 2048, 2048, "A"),
        (3, 2048, 2048, "B"),
        (5, 0, 2048, "A"),
        (4, 2048, 2048, "D"),
        (6, 0, 2048, "E"),
        (5, 2048, 1024, "B"),
        (1, 3072, 1024, "A"),
        (6, 2048, 2048, "A"),
        (7, 0, 2048, "A"),
        (5, 3072, 1024, "B"),
        (7, 2048, 1024, "D"),
        (7, 3072, 1024, "A"),
    ]


def _load_segs():
    """x_t column segments per group (non-overlapping, cover [0, XCOLS))."""
    segs = {}
    for g in range(N_GROUPS):
        if g in (0, 1):  # primer groups: small first segment
            segs[g] = [(0, 1027), (1027, 3075), (3075, XCOLS)]
        else:
            segs[g] = [(0, 2051), (2051, XCOLS)]
    return segs


def _pe_taps(path):
    if path == "A":
        return (3, 2, 1, 0)
    if path == "B":
        return (3, 2, 1)
    return ()


def _wd_layout(plan):
    """Column layout of the diag-block tensor: per PE-using group, the
    union of taps its pieces need, each tap one [CG, CG] block."""
    need = {}
    for g, _, _, path in plan:
        taps = _pe_taps(path)
        if taps:
            need.setdefault(g, set()).update(taps)
    cols = {}
    gcol = {}
    off = 0
    for g in sorted(need):
        start = off
        for k in sorted(need[g]):
            cols[(g, k)] = off
            off += CG
        gcol[g] = (start, off - start)
    return cols, gcol, off


def _build_program(plan=None, ybufs=8, tbufs=6, psbufs=4):
    import concourse.bass as bass  # noqa: F401
    import concourse.tile as tile
    from concourse import bacc, mybir

    nc = bacc.Bacc(
        "TRN2",
        target_bir_lowering=False,
        debug=False,
        enable_asserts=False,
        num_devices=N_CORES,
    )
    f32 = mybir.dt.float32
    bf16 = mybir.dt.bfloat16
    add = mybir.AluOpType.add
    mult = mybir.AluOpType.mult

    if plan is None:
        plan = _default_plan()
    wd_cols, wd_gcol, wd_ncols = _wd_layout(plan)
    segs = _load_segs()

    x_ap = nc.dram_tensor("x_t", [C, XCOLS], bf16, kind="ExternalInput").ap()
    w_ap = nc.dram_tensor("w", [CG, N_GROUPS * KTAPS], f32, kind="ExternalInput").ap()
    wd_ap = nc.dram_tensor("wd", [CG, wd_ncols], bf16, kind="ExternalInput").ap()
    out_ap = nc.dram_tensor("out", [C, T], bf16, kind="ExternalOutput").ap()

    # first plan position each PE group's wd must be resident
    first_pe_pos = {}
    for pos, (g, _, _, path) in enumerate(plan):
        if _pe_taps(path) and g not in first_pe_pos:
            first_pe_pos[g] = pos

    with tile.TileContext(nc) as tc:
        with (
            tc.tile_pool(name="wpool", bufs=1) as wpool,
            tc.tile_pool(name="xpool", bufs=1) as xpool,
            tc.tile_pool(name="ypool", bufs=ybufs) as ypool,
            tc.tile_pool(name="tpool", bufs=tbufs) as tpool,
            tc.tile_pool(name="pspool", bufs=psbufs, space="PSUM") as pspool,
        ):
            # ACT function-table preload via tiny dummy ACTIVATE
            warm = wpool.tile([CG, 1], f32)
            nc.gpsimd.memset(warm[:], 0.0)
            nc.scalar.mul(warm[:], warm[:], 1.0)

            # PE clock warmup on zeros while first loads are in flight
            wm = wpool.tile([CG, MM_N + CG], bf16)
            nc.gpsimd.memset(wm[:], 0.0)
            ps_w = pspool.tile([CG, PS_N], f32, tag="ps")
            for wi in range(4):
                nc.tensor.matmul(
                    ps_w[:, :MM_N],
                    wm[:, :CG],
                    wm[:, CG : CG + MM_N],
                    start=(wi == 0),
                    stop=(wi == 3),
                )
            nc.scalar.mul(warm[:], ps_w[:, :1], 1.0)

            wt = wpool.tile([CG, N_GROUPS * KTAPS], f32)
            wd = wpool.tile([CG, max(wd_ncols, 1)], bf16)
            # one persistent x tile per group
            xg = []
            for g in range(N_GROUPS):
                xg_t = xpool.tile([CG, XCOLS], bf16, tag=f"xg{g}", name=f"xg{g}")
                xg.append(xg_t)

            # ---- load ring (sync HWDGE, in order of first need) ----
            nc.sync.dma_start(wt[:], w_ap[:])
            wd_loaded = set()
            seg_loaded = set()

            def load_wd(g):
                if g in wd_loaded or g not in wd_gcol:
                    return
                wd_loaded.add(g)
                s, n = wd_gcol[g]
                nc.sync.dma_start(wd[:, s : s + n], wd_ap[:, s : s + n])

            def load_segs_for(g, c0, cols):
                lo, hi = c0, min(c0 + cols + HALO, XCOLS)
                r0 = g * CG
                for si, (a, b) in enumerate(segs[g]):
                    if b <= lo or a >= hi:
                        continue
                    if (g, si) in seg_loaded:
                        continue
                    seg_loaded.add((g, si))
                    nc.sync.dma_start(
                        xg[g][:, a:b], x_ap[r0 : r0 + CG, a:b]
                    )

            for pos, (g, c0, cols, path) in enumerate(plan):
                for g2, p2 in first_pe_pos.items():
                    if p2 <= pos + 2:
                        load_wd(g2)
                load_segs_for(g, c0, cols)
            for g in list(first_pe_pos):
                load_wd(g)

            def wcol(g, k):
                return g * KTAPS + k

            # ---- compute + stores ----
            for ti, (g, c0, cols, path) in enumerate(plan):
                store_eng = (
                    nc.sync if ti == len(plan) - 1
                    else nc.scalar if ti == len(plan) - 2
                    else nc.gpsimd
                )
                r0, r1 = g * CG, (g + 1) * CG
                xt = xg[g]
                yt = ypool.tile([CG, cols], bf16, tag="yt")
                if path in ("A", "B"):
                    taps = _pe_taps(path)
                    units = [
                        (u0, min(PS_N, cols - u0)) for u0 in range(0, cols, PS_N)
                    ]
                    pss = [
                        pspool.tile([CG, PS_N], f32, tag="ps", name=f"ps{ti}_{ui}")
                        for ui in range(len(units))
                    ]
                    # tap-outer across all units: stationary swaps once per
                    # tap -> matmuls pipeline at full rate; per-unit evicts
                    # drain while later taps of other units still run
                    for ki, k in enumerate(taps):
                        dcol = wd_cols[(g, k)]
                        for (u0, un), ps in zip(units, pss):
                            for m0 in range(0, un, MM_N):
                                a0 = c0 + u0 + m0
                                nc.tensor.matmul(
                                    ps[:, m0 : m0 + MM_N],
                                    wd[:, dcol : dcol + CG],
                                    xt[:, a0 + k : a0 + k + MM_N],
                                    start=(ki == 0),
                                    stop=(ki == len(taps) - 1),
                                )
                    for (u0, un), ps in zip(units, pss):
                        if path == "A":
                            nc.scalar.copy(yt[:, u0 : u0 + un], ps[:, :un])
                        else:  # B: fused tap-0 + psum add + evict on DVE
                            nc.vector.scalar_tensor_tensor(
                                yt[:, u0 : u0 + un],
                                xt[:, c0 + u0 : c0 + u0 + un],
                                wt[:, wcol(g, 0) : wcol(g, 0) + 1],
                                ps[:, :un],
                                op0=mult,
                                op1=add,
                            )
                        store_eng.dma_start(
                            out_ap[r0:r1, c0 + u0 : c0 + u0 + un],
                            yt[:, u0 : u0 + un],
                        )
                    continue  # stores already emitted per unit
                elif path == "E":
                    ta = tpool.tile([CG, cols], bf16, tag="ta")
                    tb = tpool.tile([CG, cols], bf16, tag="tb")
                    tcv = tpool.tile([CG, cols], bf16, tag="tc")
                    nc.scalar.mul(
                        ta[:], xt[:, c0 + HALO : c0 + HALO + cols],
                        wt[:, wcol(g, 3) : wcol(g, 3) + 1],
                    )
                    nc.scalar.mul(
                        tb[:], xt[:, c0 + 2 : c0 + 2 + cols],
                        wt[:, wcol(g, 2) : wcol(g, 2) + 1],
                    )
                    nc.vector.tensor_scalar_mul(
                        tcv[:], xt[:, c0 + 1 : c0 + 1 + cols],
                        wt[:, wcol(g, 1) : wcol(g, 1) + 1],
                    )
                    nc.vector.tensor_scalar_mul(
                        yt[:], xt[:, c0 : c0 + cols],
                        wt[:, wcol(g, 0) : wcol(g, 0) + 1],
                    )
                    nc.vector.tensor_tensor(ta[:], ta[:], tb[:], op=add)
                    nc.vector.tensor_tensor(yt[:], yt[:], tcv[:], op=add)
                    nc.vector.tensor_tensor(yt[:], yt[:], ta[:], op=add)
                elif path == "D":
                    ta = tpool.tile([CG, cols], bf16, tag="ta")
                    tb = tpool.tile([CG, cols], bf16, tag="tb")
                    tcv = tpool.tile([CG, cols], bf16, tag="tc")
                    nc.vector.tensor_scalar_mul(
                        ta[:], xt[:, c0 + HALO : c0 + HALO + cols],
                        wt[:, wcol(g, 3) : wcol(g, 3) + 1],
                    )
                    nc.vector.tensor_scalar_mul(
                        tb[:], xt[:, c0 + 2 : c0 + 2 + cols],
                        wt[:, wcol(g, 2) : wcol(g, 2) + 1],
                    )
                    nc.vector.tensor_scalar_mul(
                        tcv[:], xt[:, c0 + 1 : c0 + 1 + cols],
                        wt[:, wcol(g, 1) : wcol(g, 1) + 1],
                    )
                    nc.vector.tensor_scalar_mul(
                        yt[:], xt[:, c0 : c0 + cols],
                        wt[:, wcol(g, 0) : wcol(g, 0) + 1],
                    )
                    nc.vector.tensor_tensor(ta[:], ta[:], tb[:], op=add)
                    nc.vector.tensor_tensor(yt[:], yt[:], tcv[:], op=add)
                    nc.vector.tensor_tensor(yt[:], yt[:], ta[:], op=add)
                else:
                    raise ValueError(path)
                store_eng.dma_start(out_ap[r0:r1, c0 : c0 + cols], yt[:])
    nc.compile()
    return nc


def _prep_weights(W: np.ndarray) -> np.ndarray:
    # wt[p, g*KTAPS + k] = W[k, 0, g*CG + p]
    wk = W.reshape(KTAPS, N_GROUPS, CG)  # [k, g, p]
    return np.ascontiguousarray(
        wk.transpose(2, 1, 0).reshape(CG, N_GROUPS * KTAPS).astype(np.float32)
    )


def _prep_diag(W: np.ndarray, plan, bf16) -> np.ndarray:
    wd_cols, _, wd_ncols = _wd_layout(plan)
    wd = np.zeros((CG, max(wd_ncols, 1)), dtype=bf16)
    for (g, k), off in wd_cols.items():
        np.fill_diagonal(
            wd[:, off : off + CG], W[k, 0, g * CG : (g + 1) * CG].astype(bf16)
        )
    return wd


def kernel(x: np.ndarray, W: np.ndarray) -> np.ndarray:
    global last_results
    import ml_dtypes
    from concourse.bass_utils import run_bass_kernel_spmd

    bf16 = ml_dtypes.bfloat16
    x = np.asarray(x, dtype=np.float32)
    W = np.asarray(W, dtype=np.float32)
    assert x.shape == (B, T, C) and W.shape == (KTAPS, 1, C)

    plan = _default_plan()
    nc = _build_program(plan=plan)
    wt = _prep_weights(W)
    wd = _prep_diag(W, plan, bf16)
    x_bf = x.astype(bf16)
    zpad = np.zeros((C, HALO), dtype=bf16)
    in_maps = [
        {
            # [C, T+HALO] bf16, causal zero left-pad baked in
            "x_t": np.ascontiguousarray(
                np.concatenate([zpad, x_bf[i].T], axis=1)
            ),
            "w": wt,
            "wd": wd,
        }
        for i in range(N_CORES)
    ]
    import os

    trace = False
    if os.environ.get("BASS_TRACE") and not os.environ.get("BASS_NEVER_TRACE"):
        try:
            import antenv.axon_hooks  # noqa: F401

            trace = True
        except ImportError:
            os.environ["BASS_NEVER_TRACE"] = "1"
    res = run_bass_kernel_spmd(
        nc, in_maps, core_ids=list(range(N_CORES)), trace=trace
    )
    last_results = res
    y = np.stack(
        [np.asarray(res.results[i]["out"]).astype(np.float32).T for i in range(N_CORES)]
    )
    return np.ascontiguousarray(y)
